# revision 7
# baseline (speedup 1.0000x reference)
"""Nadaraya-Watson kernel regression on 8 Trainium2 NeuronCores.

reference: out[n] = sum_k softmax_k(-((q[n]-keys[n,k])*w)^2/2) * values[n,k]

Sharding: rows (N=8192) split across 8 cores, 1024 rows each; w replicated.
Per core the row softmax+reduction is fully local -> no collectives.

Math note: logits = -((q-k)w)^2/2 are all <= 0 and >= ~-50 for the given
distributions, so exp() is computed without max-subtraction; numerator and
denominator are accumulated per K-chunk and divided at the end.

Device pipeline per [128 rows x 2048 K] chunk:
  DMA   keys, values chunks (1 MB each)
  ACT   s = Square(1.0*k + (-q_row))            (per-partition bias)
  ACT   e = Exp((-w^2/2)*s), accum_out -> denom (per-partition scale)
  DVE   scalar_tensor_tensor: p = e*v, accum_out -> numer
        (tensor_tensor_reduce crashes the device on this stack; stt works)
"""

import sys

if "/opt/trn_rl_repo" not in sys.path:
    sys.path.insert(0, "/opt/trn_rl_repo")

from contextlib import ExitStack

import numpy as np

import concourse.bass as bass
import concourse.tile as tile
from concourse import bacc, mybir
from concourse.bass_utils import run_bass_kernel_spmd

N = 8192
K = 8192
N_CORES = 8
N_LOC = N // N_CORES  # 1024 rows per core
P = 128               # partitions
ROWT = N_LOC // P     # 8 row tiles per core
KC = 2048             # K chunk size
NKC = K // KC         # 4 chunks

F32 = mybir.dt.float32
AF = mybir.ActivationFunctionType
ALU = mybir.AluOpType

_cached_nc = None


def build_program(loop_iters: int | None = None) -> bass.Bass:
    """loop_iters=None: straight-line kernel. loop_iters=R: wrap the body in
    a dynamic For_i repeating the identical work R times (timing harness)."""
    nc = bacc.Bacc(
        "TRN2",
        target_bir_lowering=False,
        debug=False,
        enable_asserts=True,
        num_devices=N_CORES,
    )

    q_d = nc.dram_tensor("q", [ROWT, P, 1], F32, kind="ExternalInput")
    k_d = nc.dram_tensor("keys", [N_LOC, K], F32, kind="ExternalInput")
    v_d = nc.dram_tensor("values", [N_LOC, K], F32, kind="ExternalInput")
    wsc_d = nc.dram_tensor("wsc", [P, 1], F32, kind="ExternalInput")
    out_d = nc.dram_tensor("out", [ROWT, P, 1], F32, kind="ExternalOutput")

    with tile.TileContext(nc) as tc, ExitStack() as ctx:
        const = ctx.enter_context(tc.tile_pool(name="const", bufs=1))
        kpool = ctx.enter_context(tc.tile_pool(name="kpool", bufs=3))
        vpool = ctx.enter_context(tc.tile_pool(name="vpool", bufs=3))
        spool = ctx.enter_context(tc.tile_pool(name="spool", bufs=2))
        epool = ctx.enter_context(tc.tile_pool(name="epool", bufs=2))
        ppool = ctx.enter_context(tc.tile_pool(name="ppool", bufs=2))
        stat = ctx.enter_context(tc.tile_pool(name="stat", bufs=2))

        wsc_sb = const.tile([P, 1], F32)
        nc.sync.dma_start(wsc_sb[:], wsc_d[:])

        def body():
            for j in range(ROWT):
                qt = stat.tile([P, 1], F32, name="qt")
                nc.sync.dma_start(qt[:], q_d[j])
                qn = stat.tile([P, 1], F32, name="qn")
                nc.vector.tensor_scalar_mul(qn[:], qt[:], -1.0)

                dcols = stat.tile([P, NKC], F32, name="dcols")
                ncols = stat.tile([P, NKC], F32, name="ncols")

                for c in range(NKC):
                    kt = kpool.tile([P, KC], F32, name="kt")
                    nc.sync.dma_start(
                        kt[:], k_d[j * P:(j + 1) * P, c * KC:(c + 1) * KC]
                    )
                    vt = vpool.tile([P, KC], F32, name="vt")
                    nc.sync.dma_start(
                        vt[:], v_d[j * P:(j + 1) * P, c * KC:(c + 1) * KC]
                    )

                    st = spool.tile([P, KC], F32, name="st")
                    nc.scalar.activation(
                        st[:], kt[:], AF.Square, bias=qn[:, 0:1], scale=1.0
                    )
                    et = epool.tile([P, KC], F32, name="et")
                    nc.scalar.activation(
                        et[:], st[:], AF.Exp,
                        scale=wsc_sb[:, 0:1],
                        accum_out=dcols[:, c:c + 1],
                    )

                    pt = ppool.tile([P, KC], F32, name="pt")
                    nc.vector.scalar_tensor_tensor(
                        pt[:], et[:], 1.0, vt[:],
                        ALU.mult, ALU.mult,
                        accum_out=ncols[:, c:c + 1],
                    )

                denom = stat.tile([P, 1], F32, name="denom")
                nc.vector.tensor_reduce(denom[:], dcols[:], axis=mybir.AxisListType.X, op=ALU.add)
                numer = stat.tile([P, 1], F32, name="numer")
                nc.vector.tensor_reduce(numer[:], ncols[:], axis=mybir.AxisListType.X, op=ALU.add)
                recip = stat.tile([P, 1], F32, name="recip")
                nc.vector.reciprocal(recip[:], denom[:])
                res = stat.tile([P, 1], F32, name="res")
                nc.vector.tensor_mul(res[:], numer[:], recip[:])
                nc.sync.dma_start(out_d[j], res[:])

        if loop_iters is None:
            body()
        else:
            with tc.For_i(0, loop_iters, 1):
                body()

    if not nc.is_finalized():
        nc.finalize()
    return nc


def _run(inputs: dict, trace: bool = False):
    global _cached_nc
    if _cached_nc is None:
        _cached_nc = build_program()
    nc = _cached_nc

    queries = np.asarray(inputs["queries"], dtype=np.float32)
    keys = np.asarray(inputs["keys"], dtype=np.float32)
    values = np.asarray(inputs["values"], dtype=np.float32)
    w = np.asarray(inputs["w"], dtype=np.float32)

    wsc = np.full((P, 1), -(float(w[0]) ** 2) / 2.0, dtype=np.float32)

    in_maps = []
    for i in range(N_CORES):
        lo, hi = i * N_LOC, (i + 1) * N_LOC
        in_maps.append({
            "q": queries[lo:hi].reshape(ROWT, P, 1),
            "keys": keys[lo:hi],
            "values": values[lo:hi],
            "wsc": wsc,
        })

    res = run_bass_kernel_spmd(nc, in_maps, list(range(N_CORES)), trace=trace)
    out = np.concatenate(
        [res.results[i]["out"].reshape(N_LOC) for i in range(N_CORES)]
    ).astype(np.float32)
    return out, res


def kernel(**inputs) -> np.ndarray:
    out, _ = _run(inputs)
    return out
